# revision 41
# baseline (speedup 1.0000x reference)
"""Multi-head self-attention (N=2048, DIM=1024, NH=16, DK=64) on 8 trn2 cores.

Head-parallel sharding: core c computes heads 2c and 2c+1.
Slot-pipelined schedule: one slot per (pass, i-tile); per slot the PE emits
scores (quadrant-packed pair), ACT exps the [128,1024] score tile, and the PE
drains att for an earlier slot. The exp stream runs back-to-back and paces
the kernel. Projections are interleaved as 256-col half-chunks; K is
projected in 128-col octets so scores start as soon as the first x chunk
lands. Passes 0-2 accumulate att in the opp PSUM banks with an 8-slot lag;
pass 3 lags only 2 slots, accumulating in the projection banks (free by
then), which cuts the post-exp drain to the last att pair + one fin chain.
"""

import sys
from contextlib import ExitStack

import numpy as np

for _p in ("/opt/trn_rl_repo", "/root/.axon_site/_ro/trn_rl_repo"):
    if _p not in sys.path:
        sys.path.insert(0, _p)

import ml_dtypes  # noqa: E402

import concourse.bass as bass  # noqa: E402
import concourse.bacc as bacc  # noqa: E402
import concourse.mybir as mybir  # noqa: E402
import concourse.tile as tile  # noqa: E402
from concourse.bass import ds, ts  # noqa: E402
from concourse.bass_utils import run_bass_kernel_spmd  # noqa: E402
from concourse.masks import make_identity  # noqa: E402

N = 2048
DIM = 1024
NH = 16
DK = 64
NCORES = 8
J = 128          # head dims per core (2 heads x 64)
KT = DIM // 128  # 8 contraction tiles
MT = N // 128    # 16 m-tiles
P = 128

F32 = mybir.dt.float32
BF16 = mybir.dt.bfloat16
EXP = mybir.ActivationFunctionType.Exp

_NC_CACHE = {}


def build_nc():
    nc = bacc.Bacc("TRN2", target_bir_lowering=False, debug=False)

    # x host-packed by n-quarter: element (p, ((q*KT)+k)*512 + n) =
    # x[q*512+n, k*128+p]; quarter 0 lands first (split on two queues)
    x_d = nc.dram_tensor("xt", [P, 4 * KT * 512], BF16, kind="ExternalInput")
    # weights host-packed: element (p, k*J+j) = W^T[k*128+p, j]
    wq_d = nc.dram_tensor("wqt", [P, KT * J], BF16, kind="ExternalInput")
    wk_d = nc.dram_tensor("wkt", [P, KT * J], BF16, kind="ExternalInput")
    wv_d = nc.dram_tensor("wvt", [P, KT * J], BF16, kind="ExternalInput")
    out_d = nc.dram_tensor("out", [N, J], F32, kind="ExternalOutput")

    with tile.TileContext(nc) as tc, ExitStack() as ctx:
        pers = ctx.enter_context(tc.tile_pool(name="pers", bufs=1))
        etp = ctx.enter_context(tc.tile_pool(name="et", bufs=16))
        vnp = ctx.enter_context(tc.tile_pool(name="vn", bufs=6))
        osbp = ctx.enter_context(tc.tile_pool(name="osb", bufs=4))
        rcp = ctx.enter_context(tc.tile_pool(name="rc", bufs=4))
        outp = ctx.enter_context(tc.tile_pool(name="outp", bufs=2))
        stp = ctx.enter_context(
            tc.tile_pool(name="stp", bufs=2, space=bass.MemorySpace.PSUM)
        )
        opp = ctx.enter_context(
            tc.tile_pool(name="opp", bufs=2, space=bass.MemorySpace.PSUM)
        )
        pjp = ctx.enter_context(
            tc.tile_pool(name="pjp", bufs=1, space=bass.MemorySpace.PSUM)
        )
        ttp = ctx.enter_context(
            tc.tile_pool(name="ttp", bufs=1, space=bass.MemorySpace.PSUM)
        )

        # ---- persistent SBUF tensors
        x_sb = pers.tile([P, 4, KT, 512], BF16, tag="x")
        wq_sb = pers.tile([P, KT, J], BF16, tag="wq")
        wk_sb = pers.tile([P, KT, J], BF16, tag="wk")
        wv_sb = pers.tile([P, KT, J], BF16, tag="wv")
        qt_sb = pers.tile([P, N], BF16, tag="qt")
        kt_sb = pers.tile([P, N], BF16, tag="kt")
        vt_sb = pers.tile([P, N], BF16, tag="vt")
        vp_sb = pers.tile([P, MT, 2, DK + 1], BF16, tag="vp")
        ident = pers.tile([P, P], F32, tag="ident")
        wu_i = pers.tile([1, 1], F32, tag="wui")
        wu_o = pers.tile([1, 1], F32, tag="wuo")

        # ---- ACT exp-table warmup (overlaps the input DMA); memsets on DVE so
        # the gpsimd queue can issue its DMAs immediately
        nc.vector.memset(wu_i[:, :], 0.0)
        nc.scalar.activation(wu_o[:, :], wu_i[:, :], EXP)
        wrm = pers.tile([P, 512], BF16, tag="wrm")
        nc.vector.memset(wrm[:, :], 0.0)
        # ones column for the attention matmul (denominator trick)
        nc.vector.memset(vp_sb[:, :, :, :], 1.0)

        # ---- input DMAs. The DMA engine fair-shares bandwidth over all
        # in-flight transfers, so the chunks gating the first scores go first
        # and later chunks are held back by 1-element Pool copies that
        # complete only when the prior chunk has landed.
        qsz = KT * 512

        xgate = pers.tile([1, 8], BF16, tag="xgate")

        def gate(q):
            nc.gpsimd.tensor_copy(xgate[0:1, 0:4], x_sb[0:1, q, KT - 1, 0:4])

        nc.gpsimd.dma_start(wq_sb[:, :, :], wq_d[:, :])
        nc.sync.dma_start(x_sb[:, 0, 0:4, :], x_d[:, ds(0, qsz // 2)])
        nc.scalar.dma_start(x_sb[:, 0, 4:8, :], x_d[:, ds(qsz // 2, qsz // 2)])
        nc.gpsimd.dma_start(wk_sb[:, :, :], wk_d[:, :])
        gate(0)
        nc.gpsimd.dma_start(wv_sb[:, :, :], wv_d[:, :])
        nc.gpsimd.dma_start(x_sb[:, 1, :, :], x_d[:, ds(qsz, qsz)])
        gate(1)
        nc.gpsimd.dma_start(x_sb[:, 2, :, :], x_d[:, ds(2 * qsz, qsz)])
        gate(2)
        nc.gpsimd.dma_start(x_sb[:, 3, :, :], x_d[:, ds(3 * qsz, qsz)])

        # warm the PE (HAM un-throttle needs ~3.4us of activity) with junk
        # matmuls while the input DMA is in flight
        wps = stp.tile([P, 1024], F32, tag="st", name="warm_ps")
        for r in range(12):
            nc.tensor.matmul(
                wps[:, 0:512], wrm[:, 0:P], wrm[:, :],
                start=(r == 0), stop=(r == 11),
            )
        make_identity(nc, ident[:, :])

        _pj_alt = [0]

        def _pj_pool():
            n = _pj_alt[0]
            _pj_alt[0] += 1
            if n < 15:
                # early pass-0 items 3-way-buffer through the (still idle)
                # att-accumulator banks so the PE never waits on the DVE
                # copy of the item two back
                return ((pjp, "pj"), (ttp, "tt"), (opp, "o"))[n % 3]
            return ((pjp, "pj"), (ttp, "tt"))[n % 2]

        def project(dst_sb, w_sb, n0, half):
            """dst_sb[:, n0+half*256 : +256] = (w^T x^T) slice, k-accumulated.

            Each 512-col chunk is issued as two half-chunks (256 cols) so a
            single slot's PE burst stays under the exp cadence.
            """
            q = n0 // 512
            pool, tg = _pj_pool()
            ps = pool.tile([P, 512], F32, tag=tg,
                           name=f"pj{n0}_{half}_{_pj_alt[0]}")
            c0 = half * 256
            for k in range(KT):
                nc.tensor.matmul(
                    ps[:, ds(c0, 256)],
                    w_sb[:, k, :],
                    x_sb[:, q, k, ds(c0, 256)],
                    start=(k == 0),
                    stop=(k == KT - 1),
                )
            nc.vector.tensor_copy(
                dst_sb[:, ds(n0 + c0, 256)], ps[:, ds(c0, 256)]
            )

        def project_k_octet(o):
            """kt_sb[:, o*128 : +128]: one m-octet of the K projection (fast
            lead-in: scores(i) only needs kt up to column (i+1)*128)."""
            q, p0 = o // 4, (o % 4) * 128
            pool, tg = _pj_pool()
            ps = pool.tile([P, 512], F32, tag=tg, name=f"pko{o}")
            for k in range(KT):
                nc.tensor.matmul(
                    ps[:, ds(p0, 128)],
                    wk_sb[:, k, :],
                    x_sb[:, q, k, ds(p0, 128)],
                    start=(k == 0),
                    stop=(k == KT - 1),
                )
            nc.vector.tensor_copy(
                kt_sb[:, ds(o * 128, 128)], ps[:, ds(p0, 128)]
            )

        def vprep(i):
            """Build V' tiles for m-tile i: transpose Vt block, split heads."""
            vn = vnp.tile([P, P], BF16, tag="vn", name=f"vn{i}")
            nc.sync.dma_start_transpose(vn[:, :], vt_sb[:, ts(i, P)])
            nc.gpsimd.tensor_copy(vp_sb[:, i, 0, 0:DK], vn[:, 0:DK])
            nc.gpsimd.tensor_copy(vp_sb[:, i, 1, 0:DK], vn[:, DK:2 * DK])

        def scores_exp(i, p):
            """Scores for both heads (row-packed, concurrent) + exp."""
            n0 = p * 512
            st = stp.tile([P, 1024], F32, tag="st", name=f"st{p}_{i}")
            nc.tensor.matmul(
                st[:, 0:512],
                kt_sb[0:DK, ts(i, P)],
                qt_sb[0:DK, ds(n0, 512)],
                start=True, stop=True,
                tile_position=(0, 0),
            )
            nc.tensor.matmul(
                st[:, 512:1024],
                kt_sb[DK:2 * DK, ts(i, P)],
                qt_sb[DK:2 * DK, ds(n0, 512)],
                start=True, stop=True,
                tile_position=(64, 0),
            )
            et = etp.tile([P, 1024], BF16, tag="et", name=f"et{p}_{i}")
            nc.scalar.activation(et[:, :], st[:, :], EXP)
            return et

        def att_emit(i, o_ps, et):
            for h in range(2):
                nc.tensor.matmul(
                    o_ps[h][:, :],
                    vp_sb[:, i, h, :],
                    et[:, ds(h * 512, 512)],
                    start=(i == 0),
                    stop=(i == MT - 1),
                )

        def fin_copy(p, o_ps, osb, h, tail=False):
            """PSUM -> SBUF staging; releases an O' accumulator bank."""
            osb[h] = osbp.tile(
                [DK + 1, 512], F32, tag="osb", name=f"osb{p}_{h}"
            )
            if tail and h == 1:
                nc.scalar.copy(osb[h][:, :], o_ps[h][:, :])
            else:
                nc.vector.tensor_copy(osb[h][:, :], o_ps[h][:, :])

        def fin_tp(p, osb, h, ob):
            """Transpose head h of pass p's staging to [n, d] and divide by
            the row-sums. Passes 0/1 transpose into the projection banks;
            passes 2/3 into the freed opp banks."""
            if p < 2:
                pool, tg = _pj_pool()
            else:
                pool, tg = opp, "o"
            tt = pool.tile([P, 4, DK + 1], F32, tag=tg, name=f"tt{p}_{h}")
            for c in range(4):
                nc.tensor.transpose(
                    tt[:, c, :], osb[h][:, ts(c, P)],
                    ident[0:DK + 1, 0:DK + 1],
                )
            rcr = rcp.tile([P, 4], F32, tag="rcr", name=f"rc{p}_{h}")
            nc.vector.reciprocal(rcr[:, :], tt[:, :, DK])
            rb = rcr[:, :, None].broadcast_to([P, 4, DK])
            nc.vector.tensor_tensor(
                ob[:, :, ds(h * DK, DK)], tt[:, :, 0:DK], rb,
                op=mybir.AluOpType.mult,
            )

        def fin_dma(p, ob):
            oslc = out_d[ds(p * 512, 512), :].rearrange("(c q) j -> q c j", c=4)
            if p == 3:
                nc.gpsimd.dma_start(oslc[:, 0:1, :], ob[:, 0:1, :])
                nc.sync.dma_start(oslc[:, 1:2, :], ob[:, 1:2, :])
                nc.scalar.dma_start(oslc[:, 2:3, :], ob[:, 2:3, :])
                nc.gpsimd.dma_start(oslc[:, 3:4, :], ob[:, 3:4, :])
            else:
                nc.gpsimd.dma_start(oslc[:, 0:2, :], ob[:, 0:2, :])
                nc.sync.dma_start(oslc[:, 2:4, :], ob[:, 2:4, :])

        # ---- lead-in: just enough projection for slot 0 to start
        project(qt_sb, wq_sb, 0, 0)
        project_k_octet(0)
        project(qt_sb, wq_sb, 0, 1)

        # per-slot inserted projection/vprep work, keyed by global slot.
        # pko o gates scores(i=o) at slot o (issue at slot <= o-1); vp(i)
        # gates att(0, i) at slot i+8; pq chunk c gates pass c's scores
        # (slot 16c); pv chunk gates its vpreps.
        inserts = {
            0: [("pko", 1, 0), ("pv", 0, 0), ("vp", 0, 0), ("vp", 1, 0)],
            1: [("pko", 2, 0), ("pv", 0, 1), ("vp", 2, 0), ("vp", 3, 0)],
            2: [("pko", 3, 0), ("pko", 4, 0)],
            3: [("pko", 5, 0)],
            4: [("pko", 6, 0)],
            5: [("pko", 7, 0), ("pv", 512, 0)],
            6: [("pko", 8, 0), ("pv", 512, 1)],
            7: [("pko", 9, 0), ("pq", 512, 0), ("vp", 4, 0), ("vp", 5, 0)],
            8: [("pko", 10, 0), ("pq", 512, 1), ("vp", 6, 0), ("vp", 7, 0)],
            9: [("pko", 11, 0), ("pv", 1024, 0)],
            10: [("pko", 12, 0), ("pv", 1024, 1)],
            11: [("pko", 13, 0), ("vp", 8, 0), ("vp", 9, 0)],
            12: [("pko", 14, 0), ("vp", 10, 0), ("vp", 11, 0)],
            13: [("pko", 15, 0)],
            16: [("pv", 1536, 0)],
            17: [("pv", 1536, 1), ("vp", 12, 0), ("vp", 13, 0)],
            18: [("vp", 14, 0), ("vp", 15, 0)],
            20: [("pq", 1024, 0)],
            21: [("pq", 1024, 1)],
            36: [("pq", 1536, 0)],
            37: [("pq", 1536, 1)],
        }

        def do_insert(item):
            kind, a, b = item
            if kind == "pq":
                project(qt_sb, wq_sb, a, b)
            elif kind == "pv":
                project(vt_sb, wv_sb, a, b)
            elif kind == "pko":
                project_k_octet(a)
            else:  # vp
                vprep(a)

        # att schedule: passes 0-2 at lag 8; pass 3 at lag 2 (its last two
        # i-tiles land right after the final exp)
        att_sched = {}
        for pa in range(3):
            for ia in range(MT):
                # defer ia 0-1 by two slots: att(pa, 0) would otherwise
                # head-of-line block the PE on fin_copy(pa-1) (DVE queue)
                s = 16 * pa + 10 + ia if ia < 2 else 16 * pa + 8 + ia
                att_sched.setdefault(s, []).append((pa, ia))
        for ia in range(MT):
            att_sched.setdefault(min(50 + ia, 63 + (ia == 15)), []).append((3, ia))

        # fin transpose/DMA slots per pass (pass 3 handled in the tail slot)
        fin_sched = {
            38: [("tp", 0, 0)],
            39: [("tp", 0, 1), ("dma", 0, 0)],
            41: [("tp", 1, 0)],
            42: [("tp", 1, 1), ("dma", 1, 0)],
            56: [("tp", 2, 0)],
            57: [("tp", 2, 1), ("dma", 2, 0)],
            65: [("tp", 3, 0), ("tp", 3, 1), ("dma", 3, 0)],
        }

        osbs = [[None, None] for _ in range(4)]
        obs = [None] * 4
        att_acc = {}
        ets = {}

        for s in range(66):
            p, i = s // 16, s % 16
            if s < 64:
                ets[s] = scores_exp(i, p)
            for pa, ia in att_sched.get(s, []):
                if ia == 0:
                    if pa < 3:
                        att_acc[pa] = [
                            opp.tile([DK + 1, 512], F32, tag="o",
                                     name=f"o{pa}_{h}")
                            for h in range(2)
                        ]
                    else:
                        att_acc[pa] = [
                            pjp.tile([DK + 1, 512], F32, tag="pj",
                                     name="o3_0"),
                            ttp.tile([DK + 1, 512], F32, tag="tt",
                                     name="o3_1"),
                        ]
                att_emit(ia, att_acc[pa], ets.pop(16 * pa + ia))
                if ia == 15:
                    tail = pa == 3
                    fin_copy(pa, att_acc[pa], osbs[pa], 0, tail)
                    fin_copy(pa, att_acc[pa], osbs[pa], 1, tail)
            for ev in fin_sched.get(s, []):
                kind, pa, h = ev
                if kind == "tp":
                    if h == 0:
                        obs[pa] = outp.tile(
                            [P, 4, P], F32, tag="ob", name=f"ob{pa}"
                        )
                    fin_tp(pa, osbs[pa], h, obs[pa])
                else:
                    fin_dma(pa, obs[pa])
            for item in inserts.get(s, []):
                do_insert(item)
        assert not ets

    nc.finalize()
    return nc


def make_in_maps(x, Wq, Wk, Wv):
    x = np.asarray(x, dtype=np.float32)
    Wq = np.asarray(Wq, dtype=np.float32)
    Wk = np.asarray(Wk, dtype=np.float32)
    Wv = np.asarray(Wv, dtype=np.float32)

    bf16 = ml_dtypes.bfloat16
    scale = 1.0 / np.sqrt(DK)
    # [P, 4*KT*512]: element (p, (q*KT+k)*512+n) = x[q*512+n, k*128+p]
    xt = (x.T.reshape(KT, P, 4, 512).transpose(1, 2, 0, 3)
          .reshape(P, 4 * KT * 512))
    xt = np.ascontiguousarray(xt).astype(bf16)

    def pack_w(w_slice):
        # [DIM, J] -> [P, KT*J]: element (p, k*J+j) = W^T[k*P+p, j]
        wt = w_slice.T.reshape(KT, P, J).transpose(1, 0, 2).reshape(P, KT * J)
        return np.ascontiguousarray(wt).astype(bf16)

    in_maps = []
    for c in range(NCORES):
        sl = slice(c * J, (c + 1) * J)
        in_maps.append({
            "xt": xt,
            "wqt": pack_w(Wq[sl, :] * scale),
            "wkt": pack_w(Wk[sl, :]),
            "wvt": pack_w(Wv[sl, :]),
        })
    return in_maps


def kernel(x, rela, Wq, Wk, Wv):
    in_maps = make_in_maps(x, Wq, Wk, Wv)
    if "nc" not in _NC_CACHE:
        _NC_CACHE["nc"] = build_nc()
    res = run_bass_kernel_spmd(_NC_CACHE["nc"], in_maps, core_ids=list(range(NCORES)))
    out = np.concatenate([res.results[c]["out"] for c in range(NCORES)], axis=1)
    return np.ascontiguousarray(out.astype(np.float32))


if __name__ == "__main__":
    rng = np.random.default_rng(0)
    x = rng.standard_normal((N, DIM), dtype=np.float32)
    b = 1.0 / np.sqrt(DIM)
    Wq = rng.uniform(-b, b, (DIM, DIM)).astype(np.float32)
    Wk = rng.uniform(-b, b, (DIM, DIM)).astype(np.float32)
    Wv = rng.uniform(-b, b, (DIM, DIM)).astype(np.float32)
    out = kernel(x, np.zeros(1, np.float32), Wq, Wk, Wv)
    print(out.shape, out.dtype)


# revision 42
# speedup vs baseline: 1.0324x; 1.0324x over previous
"""Multi-head self-attention (N=2048, DIM=1024, NH=16, DK=64) on 8 trn2 cores.

Head-parallel sharding: core c computes heads 2c and 2c+1.
Slot-pipelined schedule: one slot per (pass, i-tile); per slot the PE emits
scores (quadrant-packed pair), ACT exps the [128,1024] score tile, and the PE
drains att for an earlier slot. The exp stream runs back-to-back and paces
the kernel. Projections are interleaved as 256-col half-chunks; K is
projected in 128-col octets so scores start as soon as the first x chunk
lands. Passes 0-2 accumulate att in the opp PSUM banks with an 8-slot lag;
pass 3 lags only 2 slots, accumulating in the projection banks (free by
then), which cuts the post-exp drain to the last att pair + one fin chain.
"""

import sys
from contextlib import ExitStack

import numpy as np

for _p in ("/opt/trn_rl_repo", "/root/.axon_site/_ro/trn_rl_repo"):
    if _p not in sys.path:
        sys.path.insert(0, _p)

import ml_dtypes  # noqa: E402

import concourse.bass as bass  # noqa: E402
import concourse.bacc as bacc  # noqa: E402
import concourse.mybir as mybir  # noqa: E402
import concourse.tile as tile  # noqa: E402
from concourse.bass import ds, ts  # noqa: E402
from concourse.bass_utils import run_bass_kernel_spmd  # noqa: E402
from concourse.masks import make_identity  # noqa: E402

N = 2048
DIM = 1024
NH = 16
DK = 64
NCORES = 8
J = 128          # head dims per core (2 heads x 64)
KT = DIM // 128  # 8 contraction tiles
MT = N // 128    # 16 m-tiles
P = 128

F32 = mybir.dt.float32
BF16 = mybir.dt.bfloat16
EXP = mybir.ActivationFunctionType.Exp

_NC_CACHE = {}


def build_nc():
    nc = bacc.Bacc("TRN2", target_bir_lowering=False, debug=False)

    # x host-packed by n-quarter: element (p, ((q*KT)+k)*512 + n) =
    # x[q*512+n, k*128+p]; quarter 0 lands first (split on two queues)
    x_d = nc.dram_tensor("xt", [P, 4 * KT * 512], BF16, kind="ExternalInput")
    # weights host-packed: element (p, k*J+j) = W^T[k*128+p, j]
    wq_d = nc.dram_tensor("wqt", [P, KT * J], BF16, kind="ExternalInput")
    wk_d = nc.dram_tensor("wkt", [P, KT * J], BF16, kind="ExternalInput")
    wv_d = nc.dram_tensor("wvt", [P, KT * J], BF16, kind="ExternalInput")
    out_d = nc.dram_tensor("out", [N, J], F32, kind="ExternalOutput")

    with tile.TileContext(nc) as tc, ExitStack() as ctx:
        pers = ctx.enter_context(tc.tile_pool(name="pers", bufs=1))
        etp = ctx.enter_context(tc.tile_pool(name="et", bufs=16))
        vnp = ctx.enter_context(tc.tile_pool(name="vn", bufs=6))
        osbp = ctx.enter_context(tc.tile_pool(name="osb", bufs=4))
        rcp = ctx.enter_context(tc.tile_pool(name="rc", bufs=4))
        outp = ctx.enter_context(tc.tile_pool(name="outp", bufs=2))
        stp = ctx.enter_context(
            tc.tile_pool(name="stp", bufs=2, space=bass.MemorySpace.PSUM)
        )
        opp = ctx.enter_context(
            tc.tile_pool(name="opp", bufs=2, space=bass.MemorySpace.PSUM)
        )
        pjp = ctx.enter_context(
            tc.tile_pool(name="pjp", bufs=1, space=bass.MemorySpace.PSUM)
        )
        ttp = ctx.enter_context(
            tc.tile_pool(name="ttp", bufs=1, space=bass.MemorySpace.PSUM)
        )

        # ---- persistent SBUF tensors
        x_sb = pers.tile([P, 4, KT, 512], BF16, tag="x")
        wq_sb = pers.tile([P, KT, J], BF16, tag="wq")
        wk_sb = pers.tile([P, KT, J], BF16, tag="wk")
        wv_sb = pers.tile([P, KT, J], BF16, tag="wv")
        qt_sb = pers.tile([P, N], BF16, tag="qt")
        kt_sb = pers.tile([P, N], BF16, tag="kt")
        vt_sb = pers.tile([P, N], BF16, tag="vt")
        vp_sb = pers.tile([P, MT, 2, DK + 1], BF16, tag="vp")
        ident = pers.tile([P, P], F32, tag="ident")
        wu_i = pers.tile([1, 1], F32, tag="wui")
        wu_o = pers.tile([1, 1], F32, tag="wuo")

        # ---- ACT exp-table warmup (overlaps the input DMA); memsets on DVE so
        # the gpsimd queue can issue its DMAs immediately
        nc.vector.memset(wu_i[:, :], 0.0)
        nc.scalar.activation(wu_o[:, :], wu_i[:, :], EXP)
        wrm = pers.tile([P, 512], BF16, tag="wrm")
        nc.vector.memset(wrm[:, :], 0.0)
        # ones column for the attention matmul (denominator trick)
        nc.vector.memset(vp_sb[:, :, :, :], 1.0)

        # ---- input DMAs. The DMA engine fair-shares bandwidth over all
        # in-flight transfers, so the chunks gating the first scores go first
        # and later chunks are held back by 1-element Pool copies that
        # complete only when the prior chunk has landed.
        qsz = KT * 512

        xgate = pers.tile([1, 8], BF16, tag="xgate")

        def gate(q):
            nc.gpsimd.tensor_copy(xgate[0:1, 0:4], x_sb[0:1, q, KT - 1, 0:4])

        nc.gpsimd.dma_start(wq_sb[:, :, :], wq_d[:, :])
        nc.sync.dma_start(x_sb[:, 0, 0:4, :], x_d[:, ds(0, qsz // 2)])
        nc.scalar.dma_start(x_sb[:, 0, 4:8, :], x_d[:, ds(qsz // 2, qsz // 2)])
        nc.gpsimd.dma_start(wk_sb[:, :, :], wk_d[:, :])
        gate(0)
        nc.gpsimd.dma_start(wv_sb[:, :, :], wv_d[:, :])
        nc.gpsimd.dma_start(x_sb[:, 1, :, :], x_d[:, ds(qsz, qsz)])
        gate(1)
        nc.gpsimd.dma_start(x_sb[:, 2, :, :], x_d[:, ds(2 * qsz, qsz)])
        gate(2)
        nc.gpsimd.dma_start(x_sb[:, 3, :, :], x_d[:, ds(3 * qsz, qsz)])

        # warm the PE (HAM un-throttle needs ~3.4us of activity) with junk
        # matmuls while the input DMA is in flight
        wps = stp.tile([P, 1024], F32, tag="st", name="warm_ps")
        for r in range(12):
            nc.tensor.matmul(
                wps[:, 0:512], wrm[:, 0:P], wrm[:, :],
                start=(r == 0), stop=(r == 11),
            )
        make_identity(nc, ident[:, :])

        _pj_alt = [0]

        def _pj_pool():
            n = _pj_alt[0]
            _pj_alt[0] += 1
            if n < 15:
                # early pass-0 items 3-way-buffer through the (still idle)
                # att-accumulator banks so the PE never waits on the DVE
                # copy of the item two back
                return ((pjp, "pj"), (ttp, "tt"), (opp, "o"))[n % 3]
            return ((pjp, "pj"), (ttp, "tt"))[n % 2]

        def project(dst_sb, w_sb, n0, half):
            """dst_sb[:, n0+half*256 : +256] = (w^T x^T) slice, k-accumulated.

            Each 512-col chunk is issued as two half-chunks (256 cols) so a
            single slot's PE burst stays under the exp cadence.
            """
            q = n0 // 512
            pool, tg = _pj_pool()
            ps = pool.tile([P, 512], F32, tag=tg,
                           name=f"pj{n0}_{half}_{_pj_alt[0]}")
            c0 = half * 256
            for k in range(KT):
                nc.tensor.matmul(
                    ps[:, ds(c0, 256)],
                    w_sb[:, k, :],
                    x_sb[:, q, k, ds(c0, 256)],
                    start=(k == 0),
                    stop=(k == KT - 1),
                )
            nc.vector.tensor_copy(
                dst_sb[:, ds(n0 + c0, 256)], ps[:, ds(c0, 256)]
            )

        def project_k_octet(o):
            """kt_sb[:, o*128 : +128]: one m-octet of the K projection (fast
            lead-in: scores(i) only needs kt up to column (i+1)*128)."""
            q, p0 = o // 4, (o % 4) * 128
            pool, tg = _pj_pool()
            ps = pool.tile([P, 512], F32, tag=tg, name=f"pko{o}")
            for k in range(KT):
                nc.tensor.matmul(
                    ps[:, ds(p0, 128)],
                    wk_sb[:, k, :],
                    x_sb[:, q, k, ds(p0, 128)],
                    start=(k == 0),
                    stop=(k == KT - 1),
                )
            nc.vector.tensor_copy(
                kt_sb[:, ds(o * 128, 128)], ps[:, ds(p0, 128)]
            )

        def vprep(i):
            """Build V' tiles for m-tile i: transpose Vt block, split heads."""
            vn = vnp.tile([P, P], BF16, tag="vn", name=f"vn{i}")
            nc.sync.dma_start_transpose(vn[:, :], vt_sb[:, ts(i, P)])
            nc.gpsimd.tensor_copy(vp_sb[:, i, 0, 0:DK], vn[:, 0:DK])
            nc.gpsimd.tensor_copy(vp_sb[:, i, 1, 0:DK], vn[:, DK:2 * DK])

        def scores_exp(i, p):
            """Scores for both heads (row-packed, concurrent) + exp."""
            n0 = p * 512
            st = stp.tile([P, 1024], F32, tag="st", name=f"st{p}_{i}")
            nc.tensor.matmul(
                st[:, 0:512],
                kt_sb[0:DK, ts(i, P)],
                qt_sb[0:DK, ds(n0, 512)],
                start=True, stop=True,
                tile_position=(0, 0),
            )
            nc.tensor.matmul(
                st[:, 512:1024],
                kt_sb[DK:2 * DK, ts(i, P)],
                qt_sb[DK:2 * DK, ds(n0, 512)],
                start=True, stop=True,
                tile_position=(64, 0),
            )
            et = etp.tile([P, 1024], BF16, tag="et", name=f"et{p}_{i}")
            nc.scalar.activation(et[:, :], st[:, :], EXP)
            return et

        def att_emit(i, o_ps, et):
            for h in range(2):
                nc.tensor.matmul(
                    o_ps[h][:, :],
                    vp_sb[:, i, h, :],
                    et[:, ds(h * 512, 512)],
                    start=(i == 0),
                    stop=(i == MT - 1),
                )

        def fin_copy(p, o_ps, osb, h, tail=False):
            """PSUM -> SBUF staging; releases an O' accumulator bank."""
            osb[h] = osbp.tile(
                [DK + 1, 512], F32, tag="osb", name=f"osb{p}_{h}"
            )
            if tail and h == 1:
                nc.scalar.copy(osb[h][:, :], o_ps[h][:, :])
            else:
                nc.vector.tensor_copy(osb[h][:, :], o_ps[h][:, :])

        def fin_tp(p, osb, h, ob):
            """Transpose head h of pass p's staging to [n, d] and divide by
            the row-sums. Passes 0/1 transpose into the projection banks;
            passes 2/3 into the freed opp banks."""
            if p < 2:
                pool, tg = _pj_pool()
            else:
                pool, tg = opp, "o"
            tt = pool.tile([P, 4, DK + 1], F32, tag=tg, name=f"tt{p}_{h}")
            for c in range(4):
                nc.tensor.transpose(
                    tt[:, c, :], osb[h][:, ts(c, P)],
                    ident[0:DK + 1, 0:DK + 1],
                )
            rcr = rcp.tile([P, 4], F32, tag="rcr", name=f"rc{p}_{h}")
            nc.vector.reciprocal(rcr[:, :], tt[:, :, DK])
            rb = rcr[:, :, None].broadcast_to([P, 4, DK])
            nc.vector.tensor_tensor(
                ob[:, :, ds(h * DK, DK)], tt[:, :, 0:DK], rb,
                op=mybir.AluOpType.mult,
            )

        def fin_dma(p, ob):
            oslc = out_d[ds(p * 512, 512), :].rearrange("(c q) j -> q c j", c=4)
            if p == 3:
                nc.gpsimd.dma_start(oslc[:, 0:1, :], ob[:, 0:1, :])
                nc.sync.dma_start(oslc[:, 1:2, :], ob[:, 1:2, :])
                nc.scalar.dma_start(oslc[:, 2:3, :], ob[:, 2:3, :])
                nc.gpsimd.dma_start(oslc[:, 3:4, :], ob[:, 3:4, :])
            else:
                nc.gpsimd.dma_start(oslc[:, 0:2, :], ob[:, 0:2, :])
                nc.sync.dma_start(oslc[:, 2:4, :], ob[:, 2:4, :])

        # ---- lead-in: just enough projection for slot 0 to start
        project(qt_sb, wq_sb, 0, 0)
        project_k_octet(0)
        project(qt_sb, wq_sb, 0, 1)

        # per-slot inserted projection/vprep work, keyed by global slot.
        # pko o gates scores(i=o) at slot o (issue at slot <= o-1); vp(i)
        # gates att(0, i) at slot i+8; pq chunk c gates pass c's scores
        # (slot 16c); pv chunk gates its vpreps.
        inserts = {
            0: [("pko", 1, 0), ("pv", 0, 0), ("vp", 0, 0), ("vp", 1, 0)],
            1: [("pko", 2, 0), ("pv", 0, 1), ("vp", 2, 0), ("vp", 3, 0)],
            2: [("pko", 3, 0), ("pko", 4, 0)],
            3: [("pko", 5, 0), ("pv", 512, 0)],
            4: [("pko", 6, 0), ("pv", 512, 1)],
            5: [("pko", 7, 0), ("vp", 4, 0), ("vp", 5, 0)],
            6: [("pko", 8, 0), ("vp", 6, 0), ("vp", 7, 0)],
            7: [("pko", 9, 0), ("pq", 512, 0)],
            8: [("pko", 10, 0), ("pq", 512, 1)],
            9: [("pko", 11, 0), ("pv", 1024, 0)],
            10: [("pko", 12, 0), ("pv", 1024, 1)],
            11: [("pko", 13, 0), ("vp", 8, 0), ("vp", 9, 0)],
            12: [("pko", 14, 0), ("vp", 10, 0), ("vp", 11, 0)],
            13: [("pko", 15, 0)],
            16: [("pv", 1536, 0)],
            17: [("pv", 1536, 1), ("vp", 12, 0), ("vp", 13, 0)],
            18: [("vp", 14, 0), ("vp", 15, 0)],
            20: [("pq", 1024, 0)],
            21: [("pq", 1024, 1)],
            36: [("pq", 1536, 0)],
            37: [("pq", 1536, 1)],
        }

        def do_insert(item):
            kind, a, b = item
            if kind == "pq":
                project(qt_sb, wq_sb, a, b)
            elif kind == "pv":
                project(vt_sb, wv_sb, a, b)
            elif kind == "pko":
                project_k_octet(a)
            else:  # vp
                vprep(a)

        # att schedule: passes 0-2 at lag 8; pass 3 at lag 2 (its last two
        # i-tiles land right after the final exp)
        att_sched = {}
        for pa in range(3):
            for ia in range(MT):
                # defer ia 0-1 by two slots: att(pa, 0) would otherwise
                # head-of-line block the PE on fin_copy(pa-1) (DVE queue)
                s = 16 * pa + 10 + ia if ia < 2 else 16 * pa + 8 + ia
                att_sched.setdefault(s, []).append((pa, ia))
        for ia in range(MT):
            att_sched.setdefault(min(50 + ia, 63 + (ia == 15)), []).append((3, ia))

        # fin transpose/DMA slots per pass (pass 3 handled in the tail slot)
        fin_sched = {
            38: [("tp", 0, 0)],
            39: [("tp", 0, 1), ("dma", 0, 0)],
            41: [("tp", 1, 0)],
            42: [("tp", 1, 1), ("dma", 1, 0)],
            56: [("tp", 2, 0)],
            57: [("tp", 2, 1), ("dma", 2, 0)],
            65: [("tp", 3, 0), ("tp", 3, 1), ("dma", 3, 0)],
        }

        osbs = [[None, None] for _ in range(4)]
        obs = [None] * 4
        att_acc = {}
        ets = {}

        for s in range(66):
            p, i = s // 16, s % 16
            if s < 64:
                ets[s] = scores_exp(i, p)
            for pa, ia in att_sched.get(s, []):
                if ia == 0:
                    if pa < 3:
                        att_acc[pa] = [
                            opp.tile([DK + 1, 512], F32, tag="o",
                                     name=f"o{pa}_{h}")
                            for h in range(2)
                        ]
                    else:
                        att_acc[pa] = [
                            pjp.tile([DK + 1, 512], F32, tag="pj",
                                     name="o3_0"),
                            ttp.tile([DK + 1, 512], F32, tag="tt",
                                     name="o3_1"),
                        ]
                att_emit(ia, att_acc[pa], ets.pop(16 * pa + ia))
                if ia == 15:
                    tail = pa == 3
                    fin_copy(pa, att_acc[pa], osbs[pa], 0, tail)
                    fin_copy(pa, att_acc[pa], osbs[pa], 1, tail)
            for ev in fin_sched.get(s, []):
                kind, pa, h = ev
                if kind == "tp":
                    if h == 0:
                        obs[pa] = outp.tile(
                            [P, 4, P], F32, tag="ob", name=f"ob{pa}"
                        )
                    fin_tp(pa, osbs[pa], h, obs[pa])
                else:
                    fin_dma(pa, obs[pa])
            for item in inserts.get(s, []):
                do_insert(item)
        assert not ets

    nc.finalize()
    return nc


def make_in_maps(x, Wq, Wk, Wv):
    x = np.asarray(x, dtype=np.float32)
    Wq = np.asarray(Wq, dtype=np.float32)
    Wk = np.asarray(Wk, dtype=np.float32)
    Wv = np.asarray(Wv, dtype=np.float32)

    bf16 = ml_dtypes.bfloat16
    scale = 1.0 / np.sqrt(DK)
    # [P, 4*KT*512]: element (p, (q*KT+k)*512+n) = x[q*512+n, k*128+p]
    xt = (x.T.reshape(KT, P, 4, 512).transpose(1, 2, 0, 3)
          .reshape(P, 4 * KT * 512))
    xt = np.ascontiguousarray(xt).astype(bf16)

    def pack_w(w_slice):
        # [DIM, J] -> [P, KT*J]: element (p, k*J+j) = W^T[k*P+p, j]
        wt = w_slice.T.reshape(KT, P, J).transpose(1, 0, 2).reshape(P, KT * J)
        return np.ascontiguousarray(wt).astype(bf16)

    in_maps = []
    for c in range(NCORES):
        sl = slice(c * J, (c + 1) * J)
        in_maps.append({
            "xt": xt,
            "wqt": pack_w(Wq[sl, :] * scale),
            "wkt": pack_w(Wk[sl, :]),
            "wvt": pack_w(Wv[sl, :]),
        })
    return in_maps


def kernel(x, rela, Wq, Wk, Wv):
    in_maps = make_in_maps(x, Wq, Wk, Wv)
    if "nc" not in _NC_CACHE:
        _NC_CACHE["nc"] = build_nc()
    res = run_bass_kernel_spmd(_NC_CACHE["nc"], in_maps, core_ids=list(range(NCORES)))
    out = np.concatenate([res.results[c]["out"] for c in range(NCORES)], axis=1)
    return np.ascontiguousarray(out.astype(np.float32))


if __name__ == "__main__":
    rng = np.random.default_rng(0)
    x = rng.standard_normal((N, DIM), dtype=np.float32)
    b = 1.0 / np.sqrt(DIM)
    Wq = rng.uniform(-b, b, (DIM, DIM)).astype(np.float32)
    Wk = rng.uniform(-b, b, (DIM, DIM)).astype(np.float32)
    Wv = rng.uniform(-b, b, (DIM, DIM)).astype(np.float32)
    out = kernel(x, np.zeros(1, np.float32), Wq, Wk, Wv)
    print(out.shape, out.dtype)
